# revision 99
# baseline (speedup 1.0000x reference)
"""Llama attention layer on 8 TRN2 NeuronCores.

Sharding: core = (batch b in 0..1) x (head-group g in 0..3), 4 heads each.
Per core: full hidden_states[b] (transposed on host), column slices of
wq/wk/wv, row slice of wo.T. Host sums the 4 per-head-group o_proj
partials per batch.

v3 schedule: fp8e4 DoubleRow (0.5 cycles/row, 256-deep contraction per
instruction) for the K=2048 projections and the o_proj, with exact
hi+lo e4m3 splits of both operands (3 cross terms -> bf16-grade
accuracy at 0.75x the bf16 PE cost). Scores and attn@V keep bf16
(K=128 per tile: DoubleRow buys nothing at equal accuracy there).

  - inputs arrive as hi/lo e4m3 pairs, k-pair-packed for DoubleRow
    ([128, 2, *] slices); wv + x pairs drive the startup drip with a
    dummy-matmul stream holding the PE p-state ramp.
  - v projections run in two kp-rounds: round 1 (kp0..3) pipes all 16
    seq-tiles through the PSUM banks during the x drip with bf16
    partial spills; head-0 q/k runs kp-outer with a group-major kp6/7
    tail so its group STOPs stagger; round 2 (kp4..7) chases the
    serial DVE rope chain bank-by-bank and merges partials via
    ACT-copy + Pool-add early (DVE busy roping) or a fused DVE op late.
  - scales: wq/wk/wo carry 2^5, wv 2^4 (e4m3 range); cos/sin tables
    carry 2^-5; 1/sqrt(128) moves to the exp scale arg; o_proj output
    descales by 2^-9 at the PSUM->SBUF copy.
  - attention blocks of [k=128, q=1024]; exp via ACT with mask bias;
    softmax denominator via DVE bf16 parity-chains + Pool
    partition_all_reduce; normalize produces att as e4m3 hi+lo pair
    tiles (DVE mul -> ACT hi copy -> DVE sub) feeding the DoubleRow
    o_proj.
  - projections for head h+1 (and o_proj q-chunk 0 during the last
    block) interleave into the attention loops as PE filler.
"""

import numpy as np
import ml_dtypes

B, S, H, NH, HD = 2, 2048, 2048, 16, 128
G = 4            # heads per core
HG = G * HD      # 512 head-dim columns per core
KT = H // 128    # 16 contraction chunks
KP = KT // 2     # 8 contraction pair-chunks (DoubleRow)
ST = S // 128    # 16 sequence tiles of 128
QC = 1024        # attention q-chunk
NQC = S // QC    # 2 q-chunks per head
NCORES = 8
PA_LAG = 5       # emission lag (in t-iters) of pa matmuls behind scores
DUM0 = 36                # p-state warmup dummies before round 1
DUM_KP = (12,) * 8       # drip-pacing dummies per k-pair in round 1

SQ = 32.0        # wq/wk host scale (e4m3 range)
SV = 16.0        # wv host scale
SO = 32.0        # wo host scale
SO_OUT = 1.0 / (SV * SO)        # o_proj PSUM -> out descale (2^-9)
EXP_SCALE = float(1.0 / np.sqrt(np.float32(HD)))

_NC_CACHE = {}


def _ensure_path():
    import sys
    for p in ('/opt/trn_rl_repo', '/opt/pypackages'):
        if p not in sys.path:
            sys.path.append(p)


def _build_nc():
    _ensure_path()
    from contextlib import ExitStack
    import concourse.tile as tile
    from concourse import bacc, mybir, bass_isa

    bf16 = mybir.dt.bfloat16
    fp8 = mybir.dt.float8e4
    f32 = mybir.dt.float32
    EXP = mybir.ActivationFunctionType.Exp
    RADD = bass_isa.ReduceOp.add
    DR = mybir.MatmulPerfMode.DoubleRow

    nc = bacc.Bacc('TRN2', target_bir_lowering=False, debug=False)

    # host-packed hi/lo e4m3 layouts (DoubleRow k-pair packing):
    # xH*[kp, p, i, s]   = split(x.T)[(2*kp+i)*128+p, s]
    # wqH*[h, p, k, j]   = split(wq_eff.T)[k*128+p, h*128+j]   (wkH* same)
    # wvH*[kc, p, r, j]  = split(wv_eff.T)[(4*kc+r)*128+p, j]
    # woT*[h, p, j]      = split(wo_eff.T)[h*128+p, j]
    xHh = nc.dram_tensor('xHh', [KP, 128, 2, S], fp8, kind='ExternalInput')
    xHl = nc.dram_tensor('xHl', [KP, 128, 2, S], fp8, kind='ExternalInput')
    wqHh = nc.dram_tensor('wqHh', [G, 128, KT, 128], fp8, kind='ExternalInput')
    wqHl = nc.dram_tensor('wqHl', [G, 128, KT, 128], fp8, kind='ExternalInput')
    wkHh = nc.dram_tensor('wkHh', [G, 128, KT, 128], fp8, kind='ExternalInput')
    wkHl = nc.dram_tensor('wkHl', [G, 128, KT, 128], fp8, kind='ExternalInput')
    wvHh = nc.dram_tensor('wvHh', [KT // 4, 128, 4, HG], fp8, kind='ExternalInput')
    wvHl = nc.dram_tensor('wvHl', [KT // 4, 128, 4, HG], fp8, kind='ExternalInput')
    woTh = nc.dram_tensor('woTh', [G, 128, H], fp8, kind='ExternalInput')
    woTl = nc.dram_tensor('woTl', [G, 128, H], fp8, kind='ExternalInput')
    cosT = nc.dram_tensor('cosT', [HD, S], bf16, kind='ExternalInput')
    s2T = nc.dram_tensor('s2T', [HD, S], bf16, kind='ExternalInput')
    maskb = nc.dram_tensor('maskb', [128, ST], f32, kind='ExternalInput')
    out = nc.dram_tensor('out', [S, H], bf16, kind='ExternalOutput')

    with tile.TileContext(nc) as tc, ExitStack() as top:
        persist = top.enter_context(tc.tile_pool(name='persist', bufs=1))

        cos_t = persist.tile([HD, S], bf16, tag='cos', name='cos')
        s2_t = persist.tile([HD, S], bf16, tag='s2', name='s2')
        mb_t = persist.tile([128, ST], f32, tag='mb', name='mb')

        # persistent activations
        vt = [persist.tile([128, HG], bf16, tag=f'v{i}', name=f'v{i}')
              for i in range(ST)]
        # att as e4m3 hi/lo head-pair tiles for the DoubleRow o_proj
        attH = [[persist.tile([128, 2, QC], fp8, tag=f'attH{p}_{c}',
                              name=f'attH{p}_{c}') for c in range(NQC)]
                for p in range(G // 2)]
        attL = [[persist.tile([128, 2, QC], fp8, tag=f'attL{p}_{c}',
                              name=f'attL{p}_{c}') for c in range(NQC)]
                for p in range(G // 2)]

        qk_pool = top.enter_context(tc.tile_pool(name='qk', bufs=2))
        xw = top.enter_context(tc.tile_pool(name='xw', bufs=1))
        stg = top.enter_context(tc.tile_pool(name='stg', bufs=1))
        ep = top.enter_context(tc.tile_pool(name='ep', bufs=PA_LAG + 1))
        dnp = top.enter_context(tc.tile_pool(name='dnp', bufs=1))
        so_p = None   # opened at the h==G-2 swap (frees SBUF for vp1)
        # wq/wk for head 3 live in their own pool (kept through block h3c0
        # for the deferred filler units; freed with the top ExitStack).
        wqk3 = top.enter_context(tc.tile_pool(name='wqk3', bufs=1))
        wqk_cm = tc.tile_pool(name='wqk', bufs=1)   # closed after h2 proj
        wqk = wqk_cm.__enter__()
        wvp_cm = tc.tile_pool(name='wvp', bufs=1)   # closed after v proj
        wvp = wvp_cm.__enter__()
        vp1_cm = tc.tile_pool(name='vp1', bufs=1)   # v round-1 partials
        vp1_p = vp1_cm.__enter__()

        # -------- input DMAs: wv + x pairs drive the startup drip ---------
        wvh, wvl, xth, xtl = [], [], [], []
        wqh = {('q', 'h'): [None] * G, ('q', 'l'): [None] * G,
               ('k', 'h'): [None] * G, ('k', 'l'): [None] * G}

        def load_wv(kc):
            for lst, dram, sfx in ((wvh, wvHh, 'h'), (wvl, wvHl, 'l')):
                w = wvp.tile([128, 4, HG], fp8, tag=f'wv{sfx}{kc}',
                             name=f'wv{sfx}{kc}')
                nc.sync.dma_start(w[:], dram[kc])
                lst.append(w)

        def load_w0(nm):
            dh, dl = (wqHh, wqHl) if nm == 'q' else (wkHh, wkHl)
            for sfx, dram in (('h', dh), ('l', dl)):
                w = wqk.tile([128, KT, 128], fp8, tag=f'w{nm}{sfx}0',
                             name=f'w{nm}{sfx}0')
                nc.sync.dma_start(w[:], dram[0])
                wqh[(nm, sfx)][0] = w

        # order: wv kc0/kc1 + x kp0..3 feed round 1; wq0/wk0 arrive so h0
        # can chase the x kp4..7 drip kp-outer; wv kc2/3 + tables are only
        # needed by round 2 / the ropes.
        load_wv(0)
        for kp in range(KP):
            th = xw.tile([128, 2, S], fp8, tag=f'xh{kp}', name=f'xh{kp}')
            tl = xw.tile([128, 2, S], fp8, tag=f'xl{kp}', name=f'xl{kp}')
            for t, dram in ((th, xHh), (tl, xHl)):
                nc.sync.dma_start(t[:], dram[kp])
            xth.append(th)
            xtl.append(tl)
            if kp == 0:
                load_wv(1)
            elif kp == 4:
                # wq0/wk0 after the round-1 x pairs: h0 starts ~28us and
                # pulling these out of the kp0..3 window shifts the whole
                # A-phase earlier
                load_w0('q')
                load_w0('k')
            elif kp == 5:
                load_wv(2)
            elif kp == 6:
                load_wv(3)
        nc.sync.dma_start(cos_t[:], cosT[:])
        nc.sync.dma_start(s2_t[:], s2T[:])
        nc.sync.dma_start(mb_t[:], maskb[:])
        for h in range(1, G):
            for nm in ('q', 'k'):
                dh, dl = (wqHh, wqHl) if nm == 'q' else (wkHh, wkHl)
                for sfx, dram in (('h', dh), ('l', dl)):
                    pool = wqk3 if h == G - 1 else wqk
                    w = pool.tile([128, KT, 128], fp8, tag=f'w{nm}{sfx}{h}',
                                  name=f'w{nm}{sfx}{h}')
                    nc.sync.dma_start(w[:], dram[h])
                    wqh[(nm, sfx)][h] = w

        def x_ap(sfx, kp, cols):
            t = (xth if sfx == 'h' else xtl)[kp]
            return t[:, 0:2, cols.start:cols.stop]

        def wv_ap(sfx, kp):
            kc, pr = divmod(kp, 2)
            t = (wvh if sfx == 'h' else wvl)[kc]
            return t[:, 2 * pr:2 * pr + 2, :]

        # (x_sfx, w_sfx) cross terms: hi*hi, lo*hi, hi*lo
        TERMS = (('h', 'h'), ('l', 'h'), ('h', 'l'))

        qk = {'q': [None] * G, 'k': [None] * G}

        def rope(dst_tile, cs, ps):
            # dst[:, cs] = ps*cos + rot(ps)*s2  (rot: [d] <- [(d+64)%128]).
            # The two rotation muls read PSUM (cross-partition reads are
            # only legal with a PSUM operand); the cos term comes from an
            # ACT copy running in parallel, so the PSUM bank is released
            # after ~2 DVE ops instead of the full serial rope chain.
            d = dst_tile[:, cs]
            t2 = stg.tile([128, 512], bf16, tag='t2', name='t2', bufs=2)
            nc.vector.tensor_mul(t2[0:64, :], ps[64:128, :], s2_t[0:64, cs])
            nc.vector.tensor_mul(t2[64:128, :], ps[0:64, :], s2_t[64:128, cs])
            nc.scalar.copy(d, ps[:])
            nc.vector.tensor_mul(d, d, cos_t[:, cs])
            nc.vector.tensor_add(d, d, t2[:])

        # ----- startup compute: v waves first, then head-0 q/k, ropes ----
        # A dummy matmul stream on the spare PSUM bank keeps the PE
        # p-state ramp hot through the DMA drip.
        dum_sb = persist.tile([128, 256], bf16, tag='dum', name='dum')
        nc.vector.memset(dum_sb[:], 0.0)
        with tc.tile_pool(name='pp0', bufs=1, space='PSUM') as pp0:
            # bank order matters: banks are reused by the attention pools
            # in first-fit order, so allocate in rope-priority order and
            # keep the dummy/last-rope bank highest.
            wA = {si: pp0.tile([128, HG], f32, tag=f'g{si}', name=f'vps{si}')
                  for si in range(7)}
            dum_ps = pp0.tile([128, 512], f32, tag='dum', name='dum_ps')

            def dummies(n):
                for i in range(n):
                    j = i & 3
                    nc.tensor.matmul(
                        dum_ps[:, j * 128:(j + 1) * 128],
                        lhsT=dum_sb[:, 0:128], rhs=dum_sb[:, 128:256],
                        start=True, stop=True)

            def v_mm(w_ps, kp, si, start, stop):
                for ti, (xs, ws) in enumerate(TERMS):
                    nc.tensor.matmul(
                        w_ps[:],
                        lhsT=x_ap(xs, kp, slice(si * 128, (si + 1) * 128)),
                        rhs=wv_ap(ws, kp),
                        start=start and ti == 0, stop=stop and ti == 2,
                        perf_mode=DR,
                    )

            # v projections run in two kp-rounds so real PE work fills the
            # x drip: round 1 (kp 0..3) pipes all 16 si through the banks
            # with bf16 partial spills; round 2 (kp 4..7) replaces wave B,
            # chasing the rope-chain banks, and merges via ACT copy + Pool
            # add (DVE stays clear for the ropes).
            vp1 = [None] * ST

            def v_spill(si, w_ps):
                if si < 12:
                    t = vp1_p.tile([128, HG], bf16, tag=f'vp{si}',
                                   name=f'vp{si}')
                else:
                    # the es pool is idle until block h0c0 (~25us after the
                    # merges read these) -- borrow buffers for si12..15
                    t = ep.tile([128, HG], bf16, tag='e', name=f'vp{si}')
                if si & 1:
                    nc.vector.tensor_copy(t[:], w_ps[:])
                else:
                    nc.scalar.copy(t[:], w_ps[:])
                vp1[si] = t

            dummies(DUM0)
            # round 1, si 0..6: kp-outer, paced by the wv + x kp0..3 drip
            for kp in range(4):
                for si in range(7):
                    v_mm(wA[si], kp, si, kp == 0, kp == 3)
                dummies(DUM_KP[kp])
            for si in range(7):
                v_spill(si, wA[si])
            # round 1, si 7..15: full speed during the x kp4..7 drip
            for si in range(7, ST):
                w_ps = pp0.tile([128, HG], f32, tag=f'g{(si - 7) % 7}',
                                name=f'vr1_{si}')
                for kp in range(4):
                    v_mm(w_ps, kp, si, kp == 0, kp == 3)
                v_spill(si, w_ps)

            # head-0 q/k projections: kp-outer over all 8 kp; the kp4..7
            # instructions self-pace against the tail of the x drip.
            for nm in ('q', 'k'):
                qk[nm][0] = qk_pool.tile([128, S], bf16, tag=nm, name=f'{nm}0')
            groups = [('q', 0), ('q', 1), ('k', 0), ('k', 1),
                      ('k', 2), ('k', 3), ('q', 2), ('q', 3)]
            tags = [f'g{j}' for j in range(7)] + ['dum']
            g_ps = {gc: pp0.tile([128, 512], f32, tag=tags[j], name=f'qk{j}')
                    for j, gc in enumerate(groups)}

            def qk_mm(ps, nm, h, kp, cs, start, stop):
                for ti, (xs, ws) in enumerate(TERMS):
                    nc.tensor.matmul(
                        ps[:],
                        lhsT=wqh[(nm, ws)][h][:, 2 * kp:2 * kp + 2, :],
                        rhs=x_ap(xs, kp, cs),
                        start=start and ti == 0, stop=stop and ti == 2,
                        perf_mode=DR,
                    )

            # kp-outer for kp0..5, then group-major kp6..7 so the group
            # STOPs stagger: the rope chain starts ~5us earlier and v
            # round 2 chases it without waiting
            for kp in range(KP - 2):
                for nm, c in groups:
                    qk_mm(g_ps[(nm, c)], nm, 0, kp,
                          slice(c * 512, (c + 1) * 512),
                          kp == 0, False)
            for nm, c in groups:
                for kp in (KP - 2, KP - 1):
                    qk_mm(g_ps[(nm, c)], nm, 0, kp,
                          slice(c * 512, (c + 1) * 512),
                          False, kp == KP - 1)
            for nm, c in groups:
                rope(qk[nm][0], slice(c * 512, (c + 1) * 512), g_ps[(nm, c)])
            # v round 2: chases the rope chain bank-by-bank. Early merges
            # go ACT->Pool (DVE is busy with ropes); late ones are a single
            # fused DVE op reading PSUM.
            MUL = mybir.AluOpType.mult
            ADD = mybir.AluOpType.add
            for si in range(ST):
                w_ps = pp0.tile([128, HG], f32, tag=tags[si % 8],
                                name=f'vr2_{si}')
                for kp in range(4, KP):
                    v_mm(w_ps, kp, si, kp == 4, kp == KP - 1)
                if si < 8:
                    vp2 = stg.tile([128, HG], bf16, tag='vp2', name='vp2',
                                   bufs=2)
                    nc.scalar.copy(vp2[:], w_ps[:])
                    nc.gpsimd.tensor_add(vt[si][:], vp1[si][:], vp2[:])
                else:
                    nc.vector.scalar_tensor_tensor(
                        vt[si][:], w_ps[:], 1.0, vp1[si][:], MUL, ADD)
        vp1_cm.__exit__(None, None, None)
        wvp_cm.__exit__(None, None, None)
        # so/wo live in the SBUF the v-partials and wv just freed; loading
        # wo here (DMA is idle) lets the h3 pool swap disappear entirely.
        wo_t = {}
        so_cm = tc.tile_pool(name='so_p', bufs=3)
        so_p = so_cm.__enter__()
        wop_cm = tc.tile_pool(name='wop', bufs=1)
        wop = wop_cm.__enter__()
        for sfx, dram in (('h', woTh), ('l', woTl)):
            for p in range(G // 2):
                w = wop.tile([128, 2, S], fp8, tag=f'wo{sfx}{p}',
                             name=f'wo{sfx}{p}')
                nc.sync.dma_start(w[:, 0, :], dram[2 * p])
                nc.sync.dma_start(w[:, 1, :], dram[2 * p + 1])
                wo_t[(sfx, p)] = w

        # ------------- filler stream: projections for heads 1..3 ----------
        pa_cm = tc.tile_pool(name='pa_p', bufs=1, space='PSUM')
        pa_p = pa_cm.__enter__()
        ps_cm = tc.tile_pool(name='ps_p', bufs=2, space='PSUM')
        ps_p = ps_cm.__enter__()
        pp_cm = tc.tile_pool(name='pp', bufs=2, space='PSUM')
        pp = pp_cm.__enter__()
        cur_pp = [pp]
        cur_tag = ['pp']
        filler = []

        def make_proj_group(nm, h, c):
            state = {}
            cs = slice(c * 512, (c + 1) * 512)

            def unit(j0, state=state, nm=nm, h=h, c=c, cs=cs):
                if j0 == 0:
                    if qk[nm][h] is None:
                        qk[nm][h] = qk_pool.tile([128, S], bf16, tag=nm,
                                                 name=f'{nm}{h}')
                    state['ps'] = cur_pp[0].tile([128, 512], f32,
                                                 tag=cur_tag[0], name='pp')
                ps = state['ps']
                for kp in range(j0, j0 + 2):
                    qk_mm(ps, nm, h, kp, cs, kp == 0, kp == KP - 1)
                if j0 == 6:
                    rope(qk[nm][h], cs, ps)
            return [lambda j0=j0: unit(j0) for j0 in (0, 2, 4, 6)]

        for h in range(1, G):
            if h < G - 1:
                order = [(nm, c) for nm in ('q', 'k') for c in range(4)]
            else:
                # h3: the last 3 groups (k3c2/c3, q3c2) are deferred into
                # block h3c0 as PE filler; k3c2's rope lands ~t4 (needed
                # t8), k3c3's ~t8 (needed t12), q3c2 is for h3c1.
                order = [('q', 0), ('q', 1), ('k', 0), ('k', 1),
                         ('q', 3), ('k', 2), ('k', 3), ('q', 2)]
            for nm, c in order:
                filler.extend(make_proj_group(nm, h, c))

        fill_i = 0

        def emit_fillers(n):
            nonlocal fill_i
            end = min(fill_i + n, len(filler))
            while fill_i < end:
                filler[fill_i]()
                fill_i += 1

        # ---------------- attention ----------------

        so_tiles = {}

        def o_proj_group(si, nch):
            qc, r = divmod(si, ST // NQC)
            ns = slice(nch * 512, (nch + 1) * 512)
            rs = slice(r * 128, (r + 1) * 128)
            po = cur_pp[0].tile([128, 512], f32,
                                tag=cur_tag[0], name='po')
            for p in range(G // 2):
                for ti, (asfx, wsfx) in enumerate(TERMS):
                    at = (attH if asfx == 'h' else attL)[p][qc]
                    wt = wo_t[(wsfx, p)]
                    nc.tensor.matmul(
                        po[:],
                        lhsT=at[:, 0:2, rs],
                        rhs=wt[:, 0:2, ns],
                        start=(p == 0 and ti == 0),
                        stop=(p == G // 2 - 1 and ti == 2),
                        perf_mode=DR,
                    )
            if nch == 0:
                so_tiles[si] = so_p.tile([128, H], bf16, tag='so', name='so')
            so = so_tiles[si]
            if not ((si + nch) & 1):
                nc.scalar.activation(so[:, ns], po[:],
                                     mybir.ActivationFunctionType.Copy,
                                     scale=SO_OUT)
            else:
                nc.vector.tensor_scalar_mul(so[:, ns], po[:], SO_OUT)
            if si == ST - 1:
                # last row block: per-chunk DMAs so the drain after the
                # final copy is one small transfer, not the whole row
                nc.sync.dma_start(out[si * 128:(si + 1) * 128, ns],
                                  so[:, ns])
            elif nch == 3:
                nc.sync.dma_start(out[si * 128:(si + 1) * 128, :], so[:])

        for h in range(G):
            hs_ = slice(h * 128, (h + 1) * 128)
            hp, hi_ = divmod(h, 2)
            for c in range(NQC):
                if h == G - 1 and c == 1:
                    # o_proj for q-chunk 0 interleaves into this block;
                    # si==7 is held back to cover the final
                    # normalize-chain latency after the last scores.
                    emit_fillers(len(filler))    # any leftover h3 proj
                    filler[:] = [
                        (lambda si=si, nch=nch: o_proj_group(si, nch))
                        for si in range(ST // NQC - 1)
                        for nch in range(4)
                    ]
                    fill_i = 0
                pa = pa_p.tile([128, QC], f32, tag='pa', name='pa')
                acc = {}
                es = [None] * ST

                def emit_pa(t):
                    for half in range(2):
                        fs = slice(half * 512, (half + 1) * 512)
                        nc.tensor.matmul(
                            pa[:, fs], lhsT=vt[t][:, hs_], rhs=es[t][:, fs],
                            start=(t == 0), stop=(t == ST - 1),
                        )

                for t in range(ST):
                    ps = ps_p.tile([128, QC], f32, tag='ps', name='ps')
                    for half in range(2):
                        fs = slice(half * 512, (half + 1) * 512)
                        nc.tensor.matmul(
                            ps[:, fs],
                            lhsT=qk['k'][h][:, t * 128:(t + 1) * 128],
                            rhs=qk['q'][h][:, c * QC + half * 512:
                                           c * QC + (half + 1) * 512],
                            start=True, stop=True,
                        )
                    e = ep.tile([128, QC], bf16, tag='e', name='e')
                    nc.scalar.activation(e[:], ps[:], EXP,
                                         bias=mb_t[:, t:t + 1],
                                         scale=EXP_SCALE)
                    es[t] = e
                    if h == G - 1:
                        if c == 0:
                            # 12 deferred h3 units: k3c2 t0-3, k3c3 t4-7
                            # (ropes land before their scores need them),
                            # q3c2 spread over the back half
                            if t < 8 or t in (8, 10, 12, 14):
                                emit_fillers(1)
                        elif c == 1 and t >= 4:
                            emit_fillers(2)
                    else:
                        # 14 of 16 slots: 12 units total are held back
                        # for block h3c0
                        if t not in (5, 11):
                            emit_fillers(1)
                    if t >= PA_LAG:
                        emit_pa(t - PA_LAG)
                    # denominator parity chains on DVE (bf16)
                    par = t & 1
                    if t >= 2:
                        if t < 4:
                            a = dnp.tile([128, QC], bf16, tag=f'acc{par}',
                                         name=f'acc{par}')
                            acc[par] = a
                            nc.vector.tensor_add(a[:], es[t - 2][:], e[:])
                        else:
                            nc.vector.tensor_add(acc[par][:], acc[par][:],
                                                 e[:])
                for t in range(ST - PA_LAG, ST):
                    emit_pa(t)
                for half in range(2):
                    fs = slice(half * 512, (half + 1) * 512)
                    ar = dnp.tile([128, 512], f32, tag='ar', name='ar',
                                  bufs=2)
                    nc.vector.tensor_add(acc[0][:, fs], acc[0][:, fs],
                                         acc[1][:, fs])
                    nc.gpsimd.partition_all_reduce(ar[:], acc[0][:, fs],
                                                   128, RADD)
                    nc.vector.reciprocal(ar[:], ar[:])
                    attF = stg.tile([128, 512], bf16, tag='attF',
                                    name='attF', bufs=2)
                    nc.vector.tensor_mul(attF[:], pa[:, fs], ar[:])
                    nc.scalar.copy(attH[hp][c][:, hi_, fs], attF[:])
                    nc.vector.tensor_sub(attL[hp][c][:, hi_, fs], attF[:],
                                         attH[hp][c][:, hi_, fs])

            if h == G - 2:
                # h3 projections except the 12 deferred units (k3c2/c3 +
                # q3c2 spread into block h3c0). No pool swap: the deferred
                # units and the h3c1 o_proj fillers keep using pp's banks.
                emit_fillers(len(filler) - fill_i - 12)

        # ---------------- o_proj tail ----------------
        # attention PSUM pools are done: hand their banks to a deep o_proj
        # pool so the PE never waits on a so-copy to free a bank.
        emit_fillers(len(filler))
        pp_cm.__exit__(None, None, None)
        ps_cm.__exit__(None, None, None)
        pa_cm.__exit__(None, None, None)
        po2_cm = tc.tile_pool(name='po2', bufs=6, space='PSUM')
        cur_pp[0] = po2_cm.__enter__()
        cur_tag[0] = 'po'
        for nch in range(4):
            o_proj_group(ST // NQC - 1, nch)   # si 7: att[3][0], ready early
        for si in range(ST // NQC, ST):
            for nch in range(4):
                o_proj_group(si, nch)
        po2_cm.__exit__(None, None, None)
        wop_cm.__exit__(None, None, None)
        so_cm.__exit__(None, None, None)
        wqk_cm.__exit__(None, None, None)
    nc.finalize()
    return nc


def _get_nc():
    if 'nc' not in _NC_CACHE:
        _NC_CACHE['nc'] = _build_nc()
    return _NC_CACHE['nc']


_E4 = ml_dtypes.float8_e4m3


def _split8(a):
    hi = a.astype(_E4)
    lo = (a - hi.astype(np.float32)).astype(_E4)
    return hi, lo


def _prep_in_maps(hidden_states, attention_mask, wq, wk, wv, wo):
    inv = 1.0 / (10000.0 ** (np.arange(0, HD, 2, dtype=np.float32) / np.float32(HD)))
    t = np.arange(S, dtype=np.float32)
    freqs = np.outer(t, inv).astype(np.float32)          # [S, 64]
    emb = np.concatenate([freqs, freqs], axis=1)         # [S, 128]
    tbl_scale = np.float32(1.0 / SQ)
    bf = ml_dtypes.bfloat16
    cosT = (np.ascontiguousarray(np.cos(emb).T.astype(np.float32))
            * tbl_scale).astype(bf)
    sinT = np.ascontiguousarray(np.sin(emb).T.astype(np.float32))
    s2T = sinT.copy()
    s2T[:64] *= np.float32(-1.0)
    s2T = (s2T * tbl_scale).astype(bf)

    hs = np.asarray(hidden_states, dtype=np.float32)
    mask = np.asarray(attention_mask)
    wq = np.asarray(wq, dtype=np.float32)
    wk = np.asarray(wk, dtype=np.float32)
    wv = np.asarray(wv, dtype=np.float32)
    wo = np.asarray(wo, dtype=np.float32)

    def pack_x(v):            # [H,S] -> [KP, 128, 2, S]
        return np.ascontiguousarray(
            v.reshape(KP, 2, 128, S).transpose(0, 2, 1, 3))

    def pack_w(v):            # [H,HG] -> [G, 128, KT, 128] (per-head k-major)
        return np.ascontiguousarray(
            v.reshape(KT, 128, G, 128).transpose(2, 1, 0, 3))

    def pack_wv(v):           # [H,HG] -> [KT//4, 128, 4, HG]
        return np.ascontiguousarray(
            v.reshape(KT // 4, 4, 128, HG).transpose(0, 2, 1, 3))

    # x split/pack once per batch (shared by 4 cores)
    x_packed = []
    for b in range(B):
        xh, xl = _split8(hs[b].T)
        x_packed.append((pack_x(xh), pack_x(xl)))

    # weight splits per head-group (shared by the 2 batches)
    w_packed = []
    for g in range(G):
        cols = slice(g * HG, (g + 1) * HG)
        qh, ql = _split8(wq[cols, :].T * np.float32(SQ))
        kh, kl = _split8(wk[cols, :].T * np.float32(SQ))
        vh, vl = _split8(wv[cols, :].T * np.float32(SV))
        oh, ol = _split8(np.ascontiguousarray(wo[:, cols].T) * np.float32(SO))
        w_packed.append({
            'wqHh': pack_w(qh), 'wqHl': pack_w(ql),
            'wkHh': pack_w(kh), 'wkHl': pack_w(kl),
            'wvHh': pack_wv(vh), 'wvHl': pack_wv(vl),
            'woTh': np.ascontiguousarray(oh.reshape(G, 128, H)),
            'woTl': np.ascontiguousarray(ol.reshape(G, 128, H)),
        })

    in_maps = []
    for core in range(NCORES):
        b, g = divmod(core, G)
        mb = np.where(mask[b] == 0, np.float32(-1e30), np.float32(0.0))
        mbc = np.ascontiguousarray(mb.astype(np.float32).reshape(ST, 128).T)
        m = {'xHh': x_packed[b][0], 'xHl': x_packed[b][1],
             'cosT': cosT, 's2T': s2T, 'maskb': mbc}
        m.update(w_packed[g])
        in_maps.append(m)
    return in_maps


def kernel(hidden_states, attention_mask, wq, wk, wv, wo):
    _ensure_path()
    from concourse import bass_utils
    nc = _get_nc()
    in_maps = _prep_in_maps(hidden_states, attention_mask, wq, wk, wv, wo)
    res = bass_utils.run_bass_kernel_spmd(nc, in_maps, core_ids=list(range(NCORES)))
    outs = [r['out'] for r in res.results]
    full = np.empty((B, S, H), np.float32)
    for b in range(B):
        acc = outs[G * b].astype(np.float32)
        for g in range(1, G):
            acc = acc + outs[G * b + g]
        full[b] = acc
    return full


if __name__ == '__main__':
    rng = np.random.default_rng(0)
    ins = {
        'hidden_states': rng.standard_normal((B, S, H), dtype=np.float32),
        'attention_mask': np.ones((B, S), np.int32),
        'wq': rng.standard_normal((H, H), dtype=np.float32) / np.sqrt(H),
        'wk': rng.standard_normal((H, H), dtype=np.float32) / np.sqrt(H),
        'wv': rng.standard_normal((H, H), dtype=np.float32) / np.sqrt(H),
        'wo': rng.standard_normal((H, H), dtype=np.float32) / np.sqrt(H),
    }
    out = kernel(**ins)
    print('out', out.shape, out.dtype, float(np.abs(out).mean()))
